# revision 18
# baseline (speedup 1.0000x reference)
"""Trainium2 Bass kernel for ComplexGCN (3x GCNConv + 2x MHA), 8-core SPMD.

Strategy (v3): shard destination nodes across 8 cores (512 nodes/core).
The unnormalized dense adjacency shard A^T [4096 src, 512 dst] is assembled
on the host (pure index-driven scatter of the edge list, duplicates
coalesced, self loops added), packed partition-major for wide DMA lines,
and shipped as fp8; all model math runs on device:

  deg = column sums of A^T (ones-stationary matmuls) -> AllGather ->
  dinv = 1/sqrt(deg).  GCN aggregation runs feature-major with fp8
  DoubleRow matmuls (psum[feat,dst] += x_pair^T @ A^T_pair), the symmetric
  normalization folded into input/eviction scales, and the dense W
  transform applied AFTER aggregation ((A^T X) W == A^T (X W)).  MHA
  computes q/k/v for own nodes only (fp8, gain 64), AllGathers k
  (feature-major) and v (node-major pairs with a ones column per head for
  the softmax denominator), then per head streams score matmuls ->
  grouped Exp on the ACT engine (2 PSUM banks per activation, fp8 out) ->
  fp8 DoubleRow attn@V accumulation, software-pipelined so the in-order
  tensor queue never stalls on ACT.  Per-head softmax denominators use
  reciprocal_approx_fast; out-proj contracts per head (64 rows).  All
  psum evictions run on DVE so ACT does (almost) nothing but Exp.

Matmuls are emitted back-to-back so the PE array's HAM clock gate stays
at 2.4 GHz.
"""

import numpy as np

import concourse.bass as bass
import concourse.bacc as bacc
import concourse.mybir as mybir
import concourse.tile as tile
from concourse import bass_utils
from concourse.masks import make_identity

P = 128
N = 4096
NCORES = 8
NPC = N // NCORES          # 512 dst nodes per core
NST = N // P               # 32 src tiles
NSTRIP = NPC // P          # 4 own strips
DIN = 256
HID = 256
DOUT = 128
NH = 4
DH = 64

GX0 = 8.0                  # fp8 gain on dinv-scaled x0
GQKV = 64.0                # fp8 gain on q/k/v

f32 = mybir.dt.float32
bf16 = mybir.dt.bfloat16
fp8 = mybir.dt.float8e4
AF = mybir.ActivationFunctionType
ALU = mybir.AluOpType
DR = mybir.MatmulPerfMode.DoubleRow
RG = [list(range(NCORES))]

VBLK = 68                  # per-head block in v tiles: 64 v + 1 one + 3 pad
VW = NH * VBLK             # 272 cols per m-tile
KV_V_SZ = P * 2 * VW       # one v pair-tile [128, 544]
GRP = 2                    # score tiles per Exp activation (= one DR pair)
NGRP = NST // GRP


def _build_program():
    nc = bacc.Bacc("TRN2", target_bir_lowering=False, debug=False,
                   num_devices=NCORES)

    # ---- external I/O ----
    d_AT = nc.dram_tensor("ATp", [P, NST * NPC], fp8, kind="ExternalInput")
    d_x0 = nc.dram_tensor("x0p", [P, NST * DIN], bf16, kind="ExternalInput")
    d_W1 = nc.dram_tensor("W1", [DIN, HID], bf16, kind="ExternalInput")
    d_W2 = nc.dram_tensor("W2", [HID, HID], bf16, kind="ExternalInput")
    d_W3 = nc.dram_tensor("W3", [HID, DOUT], bf16, kind="ExternalInput")
    d_b1 = nc.dram_tensor("b1C", [P, 2], f32, kind="ExternalInput")
    d_b2 = nc.dram_tensor("b2C", [P, 2], f32, kind="ExternalInput")
    d_b3 = nc.dram_tensor("b3C", [P, 1], f32, kind="ExternalInput")
    d_ipw = nc.dram_tensor("ipw", [HID, 3 * HID], bf16, kind="ExternalInput")
    d_ipb = nc.dram_tensor("ipbC64", [P, 6], f32, kind="ExternalInput")
    d_opwH = nc.dram_tensor("opwH", [NH * DH, HID], bf16,
                            kind="ExternalInput")
    d_opb = nc.dram_tensor("opb64", [1, HID], bf16, kind="ExternalInput")
    d_out = nc.dram_tensor("out", [DOUT, NPC], f32, kind="ExternalOutput")

    # ---- internal DRAM for collectives ----
    d_dmy = nc.dram_tensor("dmy_loc", [NCORES], f32)
    d_dmyg = nc.dram_tensor("dmy_glob", [NCORES * NCORES], f32,
                            addr_space="Shared")
    d_degl = nc.dram_tensor("deg_loc", [NPC], f32)
    d_degg = nc.dram_tensor("deg_glob", [N], f32, addr_space="Shared")
    KV_TOT = 2 * P * NPC + 2 * KV_V_SZ
    kv_bufs = []
    for i in range(2):
        kl = nc.dram_tensor(f"kv{i}_loc", [KV_TOT], fp8)
        kg = nc.dram_tensor(f"kv{i}_glob", [NCORES, KV_TOT], fp8,
                            addr_space="Shared")
        kv_bufs.append((kl, kg))
    x_bufs = []
    for i in range(2):
        loc = nc.dram_tensor(f"x{i}_loc", [NPC, HID], fp8)
        glob = nc.dram_tensor(f"x{i}_glob", [NCORES, NPC, HID], fp8,
                              addr_space="Shared")
        x_bufs.append((loc, glob))

    with tile.TileContext(nc) as tc:
        _emit(nc, tc, d_AT, d_x0, d_W1, d_W2, d_W3, d_b1, d_b2, d_b3,
              d_ipw, d_ipb, d_opwH, d_opb, d_out,
              d_degl, d_degg, kv_bufs, x_bufs, d_dmy, d_dmyg)
    nc.compile()
    return nc


def _emit(nc, tc, d_AT, d_x0, d_W1, d_W2, d_W3, d_b1, d_b2, d_b3,
          d_ipw, d_ipb, d_opwH, d_opb, d_out,
          d_degl, d_degg, kv_bufs, x_bufs, d_dmy, d_dmyg):
    from contextlib import ExitStack
    ctx = ExitStack()
    with ctx:
        const = ctx.enter_context(tc.tile_pool(name="const", bufs=1))
        big = ctx.enter_context(tc.tile_pool(name="big", bufs=1))
        work = ctx.enter_context(tc.tile_pool(name="work", bufs=2))
        psum = ctx.enter_context(tc.tile_pool(name="psum", bufs=1,
                                              space="PSUM"))

        def ps3():
            return psum.tile([P, GRP * NPC], f32, name="ps3", bufs=2)

        def ps_pat():
            return psum.tile([DH + 1, NPC], f32, name="pat", bufs=2)

        def ps_misc(shape, dt):
            return psum.tile(shape, dt, name="misc", bufs=1)

        # ---------------- constants ----------------
        ident_f = const.tile([P, P], f32, name="ident_f")
        make_identity(nc, ident_f[:])
        ident_b = const.tile([P, P], bf16, name="ident_b")
        make_identity(nc, ident_b[:])
        ones_col8 = const.tile([P, 1], fp8, name="ones_col8")
        nc.vector.memset(ones_col8[:], 1.0)
        ones_row_b = const.tile([1, P], bf16, name="ones_row_b")
        nc.vector.memset(ones_row_b[:], 1.0)
        ones_row_f = const.tile([1, P], f32, name="ones_row_f")
        nc.vector.memset(ones_row_f[:], 1.0)
        ones64_b = const.tile([1, DH], bf16, name="ones64_b")
        nc.vector.memset(ones64_b[:], 1.0)

        # ---------------- weight loads (pre-cast on host) ----------------
        def load(dram, shape, dt, tag):
            t = const.tile(shape, dt, name=tag)
            nc.sync.dma_start(t[:], dram)
            return t

        W1t = [load(d_W1[k * P:(k + 1) * P, :], [P, HID], bf16, f"W1t{k}")
               for k in range(2)]
        W2t = [load(d_W2[k * P:(k + 1) * P, :], [P, HID], bf16, f"W2t{k}")
               for k in range(2)]
        W3t = [load(d_W3[k * P:(k + 1) * P, :], [P, DOUT], bf16, f"W3t{k}")
               for k in range(2)]
        ipw = [load(d_ipw[k * P:(k + 1) * P, :], [P, 3 * HID], bf16,
                    f"ipw{k}") for k in range(2)]
        opwH = [load(d_opwH[h * DH:(h + 1) * DH, :], [DH, HID], bf16,
                     f"opwH{h}") for h in range(NH)]
        opb = load(d_opb[:, :], [1, HID], bf16, "opb")
        b1C = load(d_b1[:, :], [P, 2], f32, "b1C")
        b2C = load(d_b2[:, :], [P, 2], f32, "b2C")
        b3C = load(d_b3[:, :], [P, 1], f32, "b3C")
        ipbC = load(d_ipb[:, :], [P, 6], f32, "ipbC")

        # ---------------- big persistent tiles ----------------
        AT_all = big.tile([P, NST * NPC], fp8, name="AT_all")
        x0_all = big.tile([P, NST * DIN], bf16, name="x0_all")
        x0s_all = big.tile([P, NST * DIN], fp8, name="x0s_all")
        xN_all = big.tile([P, NST * HID], fp8, name="xN_all")
        kT_full = [big.tile([P, N], fp8, name=f"kTf{g}") for g in range(2)]
        v_aug = [big.tile([P, 2 * VW], fp8, name=f"vaug{mp}")
                 for mp in range(NST // 2)]
        qT = [big.tile([P, NPC], fp8, name=f"qT{g}") for g in range(2)]
        ktmp = [big.tile([P, NPC], fp8, name=f"ktmp{g}") for g in range(2)]
        vT_own = [big.tile([P, NPC], bf16, name=f"vTo{g}") for g in range(2)]
        vaug_own = [big.tile([P, 2 * VW], fp8, name=f"vaugo{sp}")
                    for sp in range(2)]
        xTagg = [big.tile([P, NPC], bf16, name=f"xTagg{k}") for k in range(2)]
        xT_own = [big.tile([P, NPC], bf16, name=f"xTo{k}") for k in range(2)]
        attn_h = [big.tile([DH, NPC], bf16, name=f"attn{h}")
                  for h in range(NH)]
        x_n = [big.tile([P, HID], fp8, name=f"x_n{s}") for s in range(NSTRIP)]
        dinv_bc8 = big.tile([P, NPC], f32, name="dinv_bc8")
        dinv_bc64 = big.tile([P, NPC], f32, name="dinv_bc64")
        out_f = big.tile([DOUT, NPC], f32, name="out_f")

        deg_row = const.tile([1, NPC], f32, name="deg_row")
        sq_row = const.tile([1, NPC], f32, name="sq_row")
        dinv_row = const.tile([1, NPC], f32, name="dinv_row")
        deg_all = const.tile([P, NST], f32, name="deg_all")
        sq_all = const.tile([P, NST], f32, name="sq_all")
        dinv_all8 = const.tile([P, NST], f32, name="dinv_all8")
        dinv_nm = const.tile([P, NSTRIP], f32, name="dinv_nm")

        # ---------------- input DMAs (wide, packed on host) --------------
        # A first: the degree reduction (and its AllGather) gates GCN1.
        NCHUNK = 8
        for ch in range(NCHUNK):
            w = NST * NPC // NCHUNK
            nc.sync.dma_start(AT_all[:, ch * w:(ch + 1) * w],
                              d_AT[:, ch * w:(ch + 1) * w])
        for ch in range(4):
            w = NST * DIN // 4
            nc.sync.dma_start(x0_all[:, ch * w:(ch + 1) * w],
                              d_x0[:, ch * w:(ch + 1) * w])

        def at_t(t):
            return AT_all[:, t * NPC:(t + 1) * NPC]

        # warm up the CC mesh path while inputs stream in
        nc.gpsimd.collective_compute(
            "AllGather", ALU.bypass, replica_groups=RG,
            ins=[d_dmy[:]], outs=[d_dmyg[:]])

        # ---------------- degree + dinv ----------------
        dps = ps_misc([1, NPC], f32)
        for t in range(NST):
            nc.tensor.matmul(dps[0:1, :], lhsT=ones_col8[:, 0:1],
                             rhs=at_t(t), start=(t == 0),
                             stop=(t == NST - 1))
        nc.vector.tensor_copy(deg_row[:], dps[0:1, :])
        nc.sync.dma_start(
            d_degl.ap().rearrange("(a b) -> a b", a=1), deg_row[:])
        nc.gpsimd.collective_compute(
            "AllGather", ALU.bypass, replica_groups=RG,
            ins=[d_degl[:]], outs=[d_degg[:]])
        nc.sync.dma_start(deg_all[:],
                          d_degg.ap().rearrange("(t p) -> p t", p=P))
        nc.scalar.sqrt(sq_all[:], deg_all[:])
        nc.vector.reciprocal(dinv_all8[:], sq_all[:])
        nc.vector.tensor_scalar(dinv_all8[:], dinv_all8[:], GX0, None,
                                op0=ALU.mult)
        # own-node dinv row (local, no collective needed)
        nc.scalar.sqrt(sq_row[:], deg_row[:])
        nc.vector.reciprocal_approx_fast(dinv_row[:], sq_row[:])
        # own-node dinv p-major (for out-proj folds)
        tp = ps_misc([P, NSTRIP], f32)
        for s in range(NSTRIP):
            nc.tensor.transpose(tp[:, s:s + 1],
                                dinv_row[0:1, s * P:(s + 1) * P],
                                ident_f[0:1, 0:1])
        nc.vector.tensor_copy(dinv_nm[:], tp[:, 0:NSTRIP])
        # broadcast own dinv over partitions (f-major column scale)
        bcp = ps_misc([P, NPC], f32)
        for s in range(NSTRIP):
            nc.tensor.matmul(bcp[:, s * P:(s + 1) * P],
                             lhsT=ones_row_f[0:1, :],
                             rhs=dinv_row[0:1, s * P:(s + 1) * P],
                             start=True, stop=True)
        nc.vector.tensor_scalar(dinv_bc8[:], bcp[:], 1.0 / GX0, None,
                                op0=ALU.mult)
        nc.vector.tensor_scalar(dinv_bc64[:], bcp[:], 1.0 / GQKV, None,
                                op0=ALU.mult)

        # x0 -> fp8, scaled by GX0 * dinv[src]
        for t in range(NST):
            nc.vector.tensor_scalar(
                x0s_all[:, t * DIN:(t + 1) * DIN],
                x0_all[:, t * DIN:(t + 1) * DIN],
                dinv_all8[:, t:t + 1], None, op0=ALU.mult)

        # ---------------- helpers ----------------
        def gcn_layer(x_all, xw, Wt, bc, biasC, n_fo, evict):
            """xTagg = bc * (x^T A^T) via fp8 DoubleRow; W^T @ xTagg -> evict."""
            aps = ps3()
            for tp2 in range(NST // 2):
                xpair = x_all[:, tp2 * 2 * xw:(tp2 + 1) * 2 * xw] \
                    .rearrange("p (s x) -> p s x", s=2)
                apair = AT_all[:, tp2 * 2 * NPC:(tp2 + 1) * 2 * NPC] \
                    .rearrange("p (s x) -> p s x", s=2)
                for k in range(2):
                    nc.tensor.matmul(aps[:, k * NPC:(k + 1) * NPC],
                                     lhsT=xpair[:, :, k * P:(k + 1) * P],
                                     rhs=apair[:, :, :],
                                     start=(tp2 == 0),
                                     stop=(tp2 == NST // 2 - 1),
                                     perf_mode=DR)
            for k in range(2):
                nc.vector.scalar_tensor_tensor(
                    xTagg[k][:], aps[:, k * NPC:(k + 1) * NPC], 1.0, bc[:],
                    op0=ALU.mult, op1=ALU.mult)
            wps = ps3()
            for fo in range(n_fo):
                for fi in range(2):
                    nc.tensor.matmul(wps[:, fo * NPC:(fo + 1) * NPC],
                                     lhsT=Wt[fi][:, fo * P:(fo + 1) * P],
                                     rhs=xTagg[fi][:], start=(fi == 0),
                                     stop=(fi == 1))
            for fo in range(n_fo):
                evict(fo, wps[:, fo * NPC:(fo + 1) * NPC], biasC)

        def mha(idx):
            """xT_own (f-major) -> x_n (n-major fp8 = relu(out) * dinv)."""
            kvloc, kvglob = kv_bufs[idx]
            V_OFF = 2 * P * NPC
            # qkv for own nodes (gain 64): k, v first (feed the AG), then q
            jorder = [2, 3, 4, 5, 0, 1]
            dests = {0: qT[0], 1: qT[1], 2: ktmp[0], 3: ktmp[1],
                     4: vT_own[0], 5: vT_own[1]}
            qps = None
            for jj, j in enumerate(jorder):
                if jj % GRP == 0:
                    qps = ps3()
                sl = qps[:, (jj % GRP) * NPC:((jj % GRP) + 1) * NPC]
                for fi in range(2):
                    nc.tensor.matmul(sl, lhsT=ipw[fi][:, j * P:(j + 1) * P],
                                     rhs=xT_own[fi][:], start=(fi == 0),
                                     stop=(fi == 1))
                nc.vector.tensor_scalar(dests[j][:], sl, GQKV,
                                        ipbC[:, j:j + 1],
                                        op0=ALU.mult, op1=ALU.add)
                if j == 2 or j == 3:
                    g = j - 2
                    nc.sync.dma_start(
                        kvloc[g * P * NPC:(g + 1) * P * NPC]
                        .rearrange("(p x) -> p x", p=P), ktmp[g][:])
            # v: transpose to node-major augmented fp8 pair layout
            for k2 in range(2):
                for s in range(NSTRIP):
                    vtp = ps_misc([P, P], bf16)
                    nc.tensor.transpose(vtp[:],
                                        vT_own[k2][:, s * P:(s + 1) * P],
                                        ident_b[:])
                    sp, half = s // 2, (s % 2) * VW
                    for hh in range(2):
                        h = 2 * k2 + hh
                        o = half + h * VBLK
                        nc.vector.tensor_copy(
                            vaug_own[sp][:, o:o + DH],
                            vtp[:, hh * DH:(hh + 1) * DH])
            for sp in range(2):
                va = vaug_own[sp][:].rearrange("p (b x) -> p b x", x=VBLK)
                nc.vector.memset(va[:, :, DH:DH + 1], 1.0)
                nc.sync.dma_start(
                    kvloc[V_OFF + sp * KV_V_SZ:V_OFF + (sp + 1) * KV_V_SZ]
                    .rearrange("(p x) -> p x", p=P), vaug_own[sp][:])
            nc.gpsimd.collective_compute(
                "AllGather", ALU.bypass, replica_groups=RG,
                ins=[kvloc[:]], outs=[kvglob[:, :]])
            # unpack
            for c in range(NCORES):
                for g in range(2):
                    nc.sync.dma_start(
                        kT_full[g][:, c * NPC:(c + 1) * NPC],
                        kvglob[c, g * P * NPC:(g + 1) * P * NPC]
                        .rearrange("(p x) -> p x", p=P))
                for sp in range(2):
                    nc.sync.dma_start(
                        v_aug[c * 2 + sp][:],
                        kvglob[c, V_OFF + sp * KV_V_SZ:
                               V_OFF + (sp + 1) * KV_V_SZ]
                        .rearrange("(p x) -> p x", p=P))

            # attention: for each head PAIR, stream m tiles; the two 64-row
            # score matmuls go to different PE row groups (concurrent), one
            # Exp covers both heads, and attn@V uses fp8 DoubleRow over
            # m-pair et tiles.  et tile layout: [m0:h0|h1, m1:h0|h1].
            pend = []          # queued (et tile, pair index mp, head h, hh)
            norm_pend = []     # queued (h, rden_b) normalize chains
            pats = {}

            def flush_pat2():
                for et_t, mp, h, hh in pend:
                    va = v_aug[mp][:].rearrange("p (s x) -> p s x", s=2)
                    e4 = et_t[:].rearrange("p (mm j x) -> p mm j x",
                                           j=2, x=NPC)
                    nc.tensor.matmul(
                        pats[h][:, :],
                        lhsT=va[:, :, h * VBLK:h * VBLK + DH + 1],
                        rhs=e4[:, :, hh, :],
                        start=(mp == 0), stop=(mp == NGRP - 1),
                        perf_mode=DR)
                del pend[:]

            def flush_norm():
                for h, rden_b in norm_pend:
                    rbp = ps_misc([DH, NPC], f32)
                    nc.tensor.matmul(rbp[0:DH, :], lhsT=ones64_b[0:1, :],
                                     rhs=rden_b[:], start=True, stop=True)
                    rdb = work.tile([DH, NPC], f32, name="rdb")
                    nc.vector.tensor_copy(rdb[:], rbp[0:DH, :])
                    nc.vector.scalar_tensor_tensor(
                        attn_h[h][:], pats[h][0:DH, :], 1.0, rdb[:],
                        op0=ALU.mult, op1=ALU.mult)
                del norm_pend[:]

            escale = float(1.0 / (np.sqrt(DH) * GQKV * GQKV))
            for g in range(2):
                pats[2 * g] = ps_pat()
                pats[2 * g + 1] = ps_pat()
                et_t = None
                for m in range(NST):
                    if m % 2 == 0:
                        et_t = work.tile([P, 4 * NPC], fp8, name="et",
                                         bufs=3)
                    sps = ps3()
                    for hh in range(2):
                        r = hh * DH
                        nc.tensor.matmul(
                            sps[:, hh * NPC:(hh + 1) * NPC],
                            lhsT=kT_full[g][r:r + DH, m * P:(m + 1) * P],
                            rhs=qT[g][r:r + DH, :], start=True, stop=True)
                    if m == 2:
                        flush_norm()
                    nc.scalar.activation(
                        et_t[:, (m % 2) * 2 * NPC:((m % 2) + 1) * 2 * NPC],
                        sps[:], AF.Exp, scale=escale)
                    flush_pat2()
                    if m % 2 == 1:
                        mp = m // 2
                        pend.append((et_t, mp, 2 * g, 0))
                        pend.append((et_t, mp, 2 * g + 1, 1))
                flush_pat2()
                # normalize part A (DVE): denominator reciprocals
                for hh in range(2):
                    h = 2 * g + hh
                    dens = work.tile([1, NPC], f32, name="dens")
                    rden = work.tile([1, NPC], f32, name="rden")
                    rden_b = work.tile([1, NPC], bf16, name="rden_b")
                    nc.vector.tensor_copy(dens[:], pats[h][DH:DH + 1, :])
                    nc.vector.reciprocal_approx_fast(rden[:], dens[:])
                    nc.vector.tensor_copy(rden_b[:], rden[:])
                    norm_pend.append((h, rden_b))
            flush_norm()

            # out-proj (+bias*64) with relu, scaled by dinv/64*64, fp8 out
            for s in range(NSTRIP):
                ops = ps_misc([P, HID], f32)
                for h in range(NH):
                    nc.tensor.matmul(ops[:],
                                     lhsT=attn_h[h][:, s * P:(s + 1) * P],
                                     rhs=opwH[h][:], start=(h == 0),
                                     stop=False)
                nc.tensor.matmul(ops[:], lhsT=ones_row_b[0:1, :],
                                 rhs=opb[:], start=False, stop=True)
                nc.vector.tensor_scalar(x_n[s][:], ops[:], 0.0,
                                        dinv_nm[:, s:s + 1],
                                        op0=ALU.max, op1=ALU.mult)

        def ag_x(idx):
            xloc, xglob = x_bufs[idx]
            for s in range(NSTRIP):
                nc.sync.dma_start(xloc[s * P:(s + 1) * P, :], x_n[s][:])
            nc.gpsimd.collective_compute(
                "AllGather", ALU.bypass, replica_groups=RG,
                ins=[xloc[:, :]], outs=[xglob[:, :, :]])
            for c in range(NCORES):
                for s in range(NSTRIP):
                    t = c * NSTRIP + s
                    nc.sync.dma_start(
                        xN_all[:, t * HID:(t + 1) * HID],
                        xglob[c, s * P:(s + 1) * P, :])

        # ---------------- network ----------------
        def evict_h(fo, ps, biasC):
            nc.vector.tensor_scalar(xT_own[fo][:], ps,
                                    biasC[:, fo:fo + 1], None, op0=ALU.add)

        gcn_layer(x0s_all, DIN, W1t, dinv_bc8, b1C, 2, evict_h)
        mha(0)
        ag_x(0)

        gcn_layer(xN_all, HID, W2t, dinv_bc64, b2C, 2, evict_h)
        mha(1)
        ag_x(1)

        def evict_out(fo, ps, biasC):
            nc.scalar.activation(out_f[:], ps, AF.Sigmoid,
                                 bias=biasC[:, 0:1])
            nc.sync.dma_start(d_out[:, :], out_f[:])

        gcn_layer(xN_all, HID, W3t, dinv_bc64, b3C, 1, evict_out)


# ----------------------------------------------------------------------------
# Host-side prep: pure index manipulation / layout / dtype casts.
# ----------------------------------------------------------------------------

def _prep_inputs(node_features, edge_index, edge_weight, W1, b1, W2, b2,
                 W3, b3, in_proj_w, in_proj_b, out_proj_w, out_proj_b):
    bfl = mybir.dt.np(bf16)
    f8 = mybir.dt.np(fp8)
    rows = np.concatenate([np.asarray(edge_index[0], np.int64),
                           np.arange(N, dtype=np.int64)])
    cols = np.concatenate([np.asarray(edge_index[1], np.int64),
                           np.arange(N, dtype=np.int64)])
    w = np.concatenate([np.asarray(edge_weight, np.float32),
                        np.ones(N, np.float32)])
    A = np.zeros((N, N), np.float32)
    np.add.at(A, (rows, cols), w)
    A8 = A.astype(f8)

    x0 = np.asarray(node_features, np.float32).astype(bfl)
    x0p = np.ascontiguousarray(
        x0.reshape(NST, P, DIN).transpose(1, 0, 2).reshape(P, NST * DIN))

    asf = lambda a: np.ascontiguousarray(np.asarray(a, np.float32))
    asb = lambda a: np.ascontiguousarray(np.asarray(a, np.float32)
                                         .astype(bfl))
    common = {
        "x0p": x0p,
        "W1": asb(W1),
        "W2": asb(W2),
        "W3": asb(W3),
        "b1C": asf(np.asarray(b1, np.float32).reshape(2, P).T),
        "b2C": asf(np.asarray(b2, np.float32).reshape(2, P).T),
        "b3C": asf(np.asarray(b3, np.float32).reshape(1, P).T),
        "ipw": asb(np.asarray(in_proj_w, np.float32).T),
        "ipbC64": asf(np.asarray(in_proj_b, np.float32).reshape(6, P).T
                      * GQKV),
        "opwH": asb(np.asarray(out_proj_w, np.float32).T),
        "opb64": asb(np.asarray(out_proj_b, np.float32).reshape(1, HID)
                     * GQKV),
    }
    in_maps = []
    for c in range(NCORES):
        m = dict(common)
        Ac = A8[:, c * NPC:(c + 1) * NPC]
        m["ATp"] = np.ascontiguousarray(
            Ac.reshape(NST, P, NPC).transpose(1, 0, 2)
            .reshape(P, NST * NPC))
        in_maps.append(m)
    return in_maps


_CACHE = {}
TRACE = False
LAST_RESULTS = None


def _get_program():
    if "prog" not in _CACHE:
        _CACHE["prog"] = _build_program()
    return _CACHE["prog"]


def kernel(**inputs):
    global LAST_RESULTS
    inputs = {k: np.asarray(v) for k, v in inputs.items()}
    in_maps = _prep_inputs(**inputs)
    nc = _get_program()
    res = bass_utils.run_bass_kernel_spmd(nc, in_maps,
                                          core_ids=list(range(NCORES)),
                                          trace=TRACE)
    LAST_RESULTS = res
    out = np.concatenate(
        [np.asarray(res.results[c]["out"]).T for c in range(NCORES)], axis=0)
    return out.astype(np.float32)


# revision 20
# speedup vs baseline: 1.0085x; 1.0085x over previous
"""Trainium2 Bass kernel for ComplexGCN (3x GCNConv + 2x MHA), 8-core SPMD.

Strategy (v3): shard destination nodes across 8 cores (512 nodes/core).
The unnormalized dense adjacency shard A^T [4096 src, 512 dst] is assembled
on the host (pure index-driven scatter of the edge list, duplicates
coalesced, self loops added), packed partition-major for wide DMA lines,
and shipped as fp8; all model math runs on device:

  deg = column sums of A^T (ones-stationary matmuls) -> AllGather ->
  dinv = 1/sqrt(deg).  GCN aggregation runs feature-major with fp8
  DoubleRow matmuls (psum[feat,dst] += x_pair^T @ A^T_pair), the symmetric
  normalization folded into input/eviction scales, and the dense W
  transform applied AFTER aggregation ((A^T X) W == A^T (X W)).  MHA
  computes q/k/v for own nodes only (fp8, gain 64), AllGathers k
  (feature-major) and v (node-major pairs with a ones column per head for
  the softmax denominator), then per head streams score matmuls ->
  grouped Exp on the ACT engine (2 PSUM banks per activation, fp8 out) ->
  fp8 DoubleRow attn@V accumulation, software-pipelined so the in-order
  tensor queue never stalls on ACT.  Per-head softmax denominators use
  reciprocal_approx_fast; out-proj contracts per head (64 rows).  All
  psum evictions run on DVE so ACT does (almost) nothing but Exp.

Matmuls are emitted back-to-back so the PE array's HAM clock gate stays
at 2.4 GHz.
"""

import numpy as np

import concourse.bass as bass
import concourse.bacc as bacc
import concourse.mybir as mybir
import concourse.tile as tile
from concourse import bass_utils
from concourse.masks import make_identity

P = 128
N = 4096
NCORES = 8
NPC = N // NCORES          # 512 dst nodes per core
NST = N // P               # 32 src tiles
NSTRIP = NPC // P          # 4 own strips
DIN = 256
HID = 256
DOUT = 128
NH = 4
DH = 64

GX0 = 8.0                  # fp8 gain on dinv-scaled x0
GQKV = 64.0                # fp8 gain on q/k/v

f32 = mybir.dt.float32
bf16 = mybir.dt.bfloat16
fp8 = mybir.dt.float8e4
AF = mybir.ActivationFunctionType
ALU = mybir.AluOpType
DR = mybir.MatmulPerfMode.DoubleRow
RG = [list(range(NCORES))]

VBLK = 68                  # per-head block in v tiles: 64 v + 1 one + 3 pad
VW = NH * VBLK             # 272 cols per m-tile
KV_V_SZ = P * 2 * VW       # one v pair-tile [128, 544]
GRP = 2                    # score tiles per Exp activation (= one DR pair)
NGRP = NST // GRP


def _build_program():
    nc = bacc.Bacc("TRN2", target_bir_lowering=False, debug=False,
                   num_devices=NCORES)

    # ---- external I/O ----
    d_AT = nc.dram_tensor("ATp", [P, NST * NPC], fp8, kind="ExternalInput")
    d_x0 = nc.dram_tensor("x0p", [P, NST * DIN], bf16, kind="ExternalInput")
    d_W1 = nc.dram_tensor("W1", [DIN, HID], bf16, kind="ExternalInput")
    d_W2 = nc.dram_tensor("W2", [HID, HID], bf16, kind="ExternalInput")
    d_W3 = nc.dram_tensor("W3", [HID, DOUT], bf16, kind="ExternalInput")
    d_b1 = nc.dram_tensor("b1C", [P, 2], f32, kind="ExternalInput")
    d_b2 = nc.dram_tensor("b2C", [P, 2], f32, kind="ExternalInput")
    d_b3 = nc.dram_tensor("b3C", [P, 1], f32, kind="ExternalInput")
    d_ipw = nc.dram_tensor("ipw", [HID, 3 * HID], bf16, kind="ExternalInput")
    d_ipb = nc.dram_tensor("ipbC64", [P, 6], f32, kind="ExternalInput")
    d_opwH = nc.dram_tensor("opwH", [NH * DH, HID], bf16,
                            kind="ExternalInput")
    d_opb = nc.dram_tensor("opb64", [1, HID], bf16, kind="ExternalInput")
    d_out = nc.dram_tensor("out", [DOUT, NPC], f32, kind="ExternalOutput")

    # ---- internal DRAM for collectives ----
    d_dmy = nc.dram_tensor("dmy_loc", [NCORES], f32)
    d_dmyg = nc.dram_tensor("dmy_glob", [NCORES * NCORES], f32,
                            addr_space="Shared")
    d_degl = nc.dram_tensor("deg_loc", [NPC], f32)
    d_degg = nc.dram_tensor("deg_glob", [N], f32, addr_space="Shared")
    KV_TOT = 2 * P * NPC + 2 * KV_V_SZ
    kv_bufs = []
    for i in range(2):
        kl = nc.dram_tensor(f"kv{i}_loc", [KV_TOT], fp8)
        kg = nc.dram_tensor(f"kv{i}_glob", [NCORES, KV_TOT], fp8,
                            addr_space="Shared")
        kv_bufs.append((kl, kg))
    x_bufs = []
    for i in range(2):
        loc = nc.dram_tensor(f"x{i}_loc", [NPC, HID], fp8)
        glob = nc.dram_tensor(f"x{i}_glob", [NCORES, NPC, HID], fp8,
                              addr_space="Shared")
        x_bufs.append((loc, glob))

    with tile.TileContext(nc) as tc:
        _emit(nc, tc, d_AT, d_x0, d_W1, d_W2, d_W3, d_b1, d_b2, d_b3,
              d_ipw, d_ipb, d_opwH, d_opb, d_out,
              d_degl, d_degg, kv_bufs, x_bufs, d_dmy, d_dmyg)
    nc.compile()
    return nc


def _emit(nc, tc, d_AT, d_x0, d_W1, d_W2, d_W3, d_b1, d_b2, d_b3,
          d_ipw, d_ipb, d_opwH, d_opb, d_out,
          d_degl, d_degg, kv_bufs, x_bufs, d_dmy, d_dmyg):
    from contextlib import ExitStack
    ctx = ExitStack()
    with ctx:
        const = ctx.enter_context(tc.tile_pool(name="const", bufs=1))
        big = ctx.enter_context(tc.tile_pool(name="big", bufs=1))
        work = ctx.enter_context(tc.tile_pool(name="work", bufs=2))
        psum = ctx.enter_context(tc.tile_pool(name="psum", bufs=1,
                                              space="PSUM"))

        def ps3():
            return psum.tile([P, GRP * NPC], f32, name="ps3", bufs=2)

        def ps_pat():
            return psum.tile([DH + 1, NPC], f32, name="pat", bufs=2)

        def ps_misc(shape, dt):
            return psum.tile(shape, dt, name="misc", bufs=1)

        # ---------------- constants ----------------
        ident_f = const.tile([P, P], f32, name="ident_f")
        make_identity(nc, ident_f[:])
        ident_b = const.tile([P, P], bf16, name="ident_b")
        make_identity(nc, ident_b[:])
        ones_col8 = const.tile([P, 1], fp8, name="ones_col8")
        nc.vector.memset(ones_col8[:], 1.0)
        ones_row_b = const.tile([1, P], bf16, name="ones_row_b")
        nc.vector.memset(ones_row_b[:], 1.0)
        ones_row_f = const.tile([1, P], f32, name="ones_row_f")
        nc.vector.memset(ones_row_f[:], 1.0)
        ones64_b = const.tile([1, DH], bf16, name="ones64_b")
        nc.vector.memset(ones64_b[:], 1.0)

        # ---------------- weight loads (pre-cast on host) ----------------
        def load(dram, shape, dt, tag):
            t = const.tile(shape, dt, name=tag)
            nc.sync.dma_start(t[:], dram)
            return t

        W1t = [load(d_W1[k * P:(k + 1) * P, :], [P, HID], bf16, f"W1t{k}")
               for k in range(2)]
        W2t = [load(d_W2[k * P:(k + 1) * P, :], [P, HID], bf16, f"W2t{k}")
               for k in range(2)]
        W3t = [load(d_W3[k * P:(k + 1) * P, :], [P, DOUT], bf16, f"W3t{k}")
               for k in range(2)]
        ipw = [load(d_ipw[k * P:(k + 1) * P, :], [P, 3 * HID], bf16,
                    f"ipw{k}") for k in range(2)]
        opwH = [load(d_opwH[h * DH:(h + 1) * DH, :], [DH, HID], bf16,
                     f"opwH{h}") for h in range(NH)]
        opb = load(d_opb[:, :], [1, HID], bf16, "opb")
        b1C = load(d_b1[:, :], [P, 2], f32, "b1C")
        b2C = load(d_b2[:, :], [P, 2], f32, "b2C")
        b3C = load(d_b3[:, :], [P, 1], f32, "b3C")
        ipbC = load(d_ipb[:, :], [P, 6], f32, "ipbC")

        # ---------------- big persistent tiles ----------------
        AT_all = big.tile([P, NST * NPC], fp8, name="AT_all")
        x0_all = big.tile([P, NST * DIN], bf16, name="x0_all")
        x0s_all = big.tile([P, NST * DIN], fp8, name="x0s_all")
        xN_all = big.tile([P, NST * HID], fp8, name="xN_all")
        kT_full = [big.tile([P, N], fp8, name=f"kTf{g}") for g in range(2)]
        v_aug = [big.tile([P, 2 * VW], fp8, name=f"vaug{mp}")
                 for mp in range(NST // 2)]
        qT = [big.tile([P, NPC], fp8, name=f"qT{g}") for g in range(2)]
        ktmp = [big.tile([P, NPC], fp8, name=f"ktmp{g}") for g in range(2)]
        vT_own = [big.tile([P, NPC], bf16, name=f"vTo{g}") for g in range(2)]
        vaug_own = [big.tile([P, 2 * VW], fp8, name=f"vaugo{sp}")
                    for sp in range(2)]
        xTagg = [big.tile([P, NPC], bf16, name=f"xTagg{k}") for k in range(2)]
        xT_own = [big.tile([P, NPC], bf16, name=f"xTo{k}") for k in range(2)]
        attn_h = [big.tile([DH, NPC], bf16, name=f"attn{h}")
                  for h in range(NH)]
        x_n = [big.tile([P, HID], fp8, name=f"x_n{s}") for s in range(NSTRIP)]
        dinv_bc8 = big.tile([P, NPC], f32, name="dinv_bc8")
        dinv_bc64 = big.tile([P, NPC], f32, name="dinv_bc64")
        out_f = big.tile([DOUT, NPC], f32, name="out_f")

        deg_row = const.tile([1, NPC], f32, name="deg_row")
        sq_row = const.tile([1, NPC], f32, name="sq_row")
        dinv_row = const.tile([1, NPC], f32, name="dinv_row")
        deg_all = const.tile([P, NST], f32, name="deg_all")
        sq_all = const.tile([P, NST], f32, name="sq_all")
        dinv_all8 = const.tile([P, NST], f32, name="dinv_all8")
        dinv_nm = const.tile([P, NSTRIP], f32, name="dinv_nm")

        # ---------------- input DMAs (wide, packed on host) --------------
        # A first: the degree reduction (and its AllGather) gates GCN1.
        NCHUNK = 8
        for ch in range(NCHUNK):
            w = NST * NPC // NCHUNK
            nc.sync.dma_start(AT_all[:, ch * w:(ch + 1) * w],
                              d_AT[:, ch * w:(ch + 1) * w])
        for ch in range(4):
            w = NST * DIN // 4
            nc.sync.dma_start(x0_all[:, ch * w:(ch + 1) * w],
                              d_x0[:, ch * w:(ch + 1) * w])

        def at_t(t):
            return AT_all[:, t * NPC:(t + 1) * NPC]

        # warm up the CC mesh path while inputs stream in
        nc.gpsimd.collective_compute(
            "AllGather", ALU.bypass, replica_groups=RG,
            ins=[d_dmy[:]], outs=[d_dmyg[:]])

        # ---------------- degree + dinv ----------------
        dps = ps_misc([1, NPC], f32)
        for t in range(NST):
            nc.tensor.matmul(dps[0:1, :], lhsT=ones_col8[:, 0:1],
                             rhs=at_t(t), start=(t == 0),
                             stop=(t == NST - 1))
        nc.vector.tensor_copy(deg_row[:], dps[0:1, :])
        # scalar-engine DMA queue: dodges head-of-line blocking behind the
        # big input chunks on the sync queues
        nc.scalar.dma_start(
            d_degl.ap().rearrange("(a b) -> a b", a=1), deg_row[:])
        nc.gpsimd.collective_compute(
            "AllGather", ALU.bypass, replica_groups=RG,
            ins=[d_degl[:]], outs=[d_degg[:]])
        nc.scalar.dma_start(deg_all[:],
                            d_degg.ap().rearrange("(t p) -> p t", p=P))
        nc.scalar.sqrt(sq_all[:], deg_all[:])
        nc.vector.reciprocal(dinv_all8[:], sq_all[:])
        nc.vector.tensor_scalar(dinv_all8[:], dinv_all8[:], GX0, None,
                                op0=ALU.mult)
        # own-node dinv row (local, no collective needed)
        nc.scalar.sqrt(sq_row[:], deg_row[:])
        nc.vector.reciprocal_approx_fast(dinv_row[:], sq_row[:])
        # own-node dinv p-major (for out-proj folds)
        tp = ps_misc([P, NSTRIP], f32)
        for s in range(NSTRIP):
            nc.tensor.transpose(tp[:, s:s + 1],
                                dinv_row[0:1, s * P:(s + 1) * P],
                                ident_f[0:1, 0:1])
        nc.vector.tensor_copy(dinv_nm[:], tp[:, 0:NSTRIP])
        # broadcast own dinv over partitions (f-major column scale)
        bcp = ps_misc([P, NPC], f32)
        for s in range(NSTRIP):
            nc.tensor.matmul(bcp[:, s * P:(s + 1) * P],
                             lhsT=ones_row_f[0:1, :],
                             rhs=dinv_row[0:1, s * P:(s + 1) * P],
                             start=True, stop=True)
        nc.vector.tensor_scalar(dinv_bc8[:], bcp[:], 1.0 / GX0, None,
                                op0=ALU.mult)
        nc.vector.tensor_scalar(dinv_bc64[:], bcp[:], 1.0 / GQKV, None,
                                op0=ALU.mult)

        # x0 -> fp8, scaled by GX0 * dinv[src]
        for t in range(NST):
            nc.vector.tensor_scalar(
                x0s_all[:, t * DIN:(t + 1) * DIN],
                x0_all[:, t * DIN:(t + 1) * DIN],
                dinv_all8[:, t:t + 1], None, op0=ALU.mult)

        # ---------------- helpers ----------------
        def gcn_layer(x_all, xw, Wt, bc, biasC, n_fo, evict):
            """xTagg = bc * (x^T A^T) via fp8 DoubleRow; W^T @ xTagg -> evict."""
            aps = ps3()
            for tp2 in range(NST // 2):
                xpair = x_all[:, tp2 * 2 * xw:(tp2 + 1) * 2 * xw] \
                    .rearrange("p (s x) -> p s x", s=2)
                apair = AT_all[:, tp2 * 2 * NPC:(tp2 + 1) * 2 * NPC] \
                    .rearrange("p (s x) -> p s x", s=2)
                for k in range(2):
                    nc.tensor.matmul(aps[:, k * NPC:(k + 1) * NPC],
                                     lhsT=xpair[:, :, k * P:(k + 1) * P],
                                     rhs=apair[:, :, :],
                                     start=(tp2 == 0),
                                     stop=(tp2 == NST // 2 - 1),
                                     perf_mode=DR)
            for k in range(2):
                nc.vector.scalar_tensor_tensor(
                    xTagg[k][:], aps[:, k * NPC:(k + 1) * NPC], 1.0, bc[:],
                    op0=ALU.mult, op1=ALU.mult)
            wps = ps3()
            for fo in range(n_fo):
                for fi in range(2):
                    nc.tensor.matmul(wps[:, fo * NPC:(fo + 1) * NPC],
                                     lhsT=Wt[fi][:, fo * P:(fo + 1) * P],
                                     rhs=xTagg[fi][:], start=(fi == 0),
                                     stop=(fi == 1))
            for fo in range(n_fo):
                evict(fo, wps[:, fo * NPC:(fo + 1) * NPC], biasC)

        def mha(idx):
            """xT_own (f-major) -> x_n (n-major fp8 = relu(out) * dinv)."""
            kvloc, kvglob = kv_bufs[idx]
            V_OFF = 2 * P * NPC
            # qkv for own nodes (gain 64): k, v first (feed the AG), then q
            jorder = [2, 3, 4, 5, 0, 1]
            dests = {0: qT[0], 1: qT[1], 2: ktmp[0], 3: ktmp[1],
                     4: vT_own[0], 5: vT_own[1]}
            qps = None
            for jj, j in enumerate(jorder):
                if jj % GRP == 0:
                    qps = ps3()
                sl = qps[:, (jj % GRP) * NPC:((jj % GRP) + 1) * NPC]
                for fi in range(2):
                    nc.tensor.matmul(sl, lhsT=ipw[fi][:, j * P:(j + 1) * P],
                                     rhs=xT_own[fi][:], start=(fi == 0),
                                     stop=(fi == 1))
                nc.vector.tensor_scalar(dests[j][:], sl, GQKV,
                                        ipbC[:, j:j + 1],
                                        op0=ALU.mult, op1=ALU.add)
                if j == 2 or j == 3:
                    g = j - 2
                    nc.sync.dma_start(
                        kvloc[g * P * NPC:(g + 1) * P * NPC]
                        .rearrange("(p x) -> p x", p=P), ktmp[g][:])
            # v: transpose to node-major augmented fp8 pair layout
            for k2 in range(2):
                for s in range(NSTRIP):
                    vtp = ps_misc([P, P], bf16)
                    nc.tensor.transpose(vtp[:],
                                        vT_own[k2][:, s * P:(s + 1) * P],
                                        ident_b[:])
                    sp, half = s // 2, (s % 2) * VW
                    for hh in range(2):
                        h = 2 * k2 + hh
                        o = half + h * VBLK
                        nc.vector.tensor_copy(
                            vaug_own[sp][:, o:o + DH],
                            vtp[:, hh * DH:(hh + 1) * DH])
            for sp in range(2):
                va = vaug_own[sp][:].rearrange("p (b x) -> p b x", x=VBLK)
                nc.vector.memset(va[:, :, DH:DH + 1], 1.0)
                nc.sync.dma_start(
                    kvloc[V_OFF + sp * KV_V_SZ:V_OFF + (sp + 1) * KV_V_SZ]
                    .rearrange("(p x) -> p x", p=P), vaug_own[sp][:])
            nc.gpsimd.collective_compute(
                "AllGather", ALU.bypass, replica_groups=RG,
                ins=[kvloc[:]], outs=[kvglob[:, :]])
            # unpack
            for c in range(NCORES):
                for g in range(2):
                    nc.sync.dma_start(
                        kT_full[g][:, c * NPC:(c + 1) * NPC],
                        kvglob[c, g * P * NPC:(g + 1) * P * NPC]
                        .rearrange("(p x) -> p x", p=P))
                for sp in range(2):
                    nc.sync.dma_start(
                        v_aug[c * 2 + sp][:],
                        kvglob[c, V_OFF + sp * KV_V_SZ:
                               V_OFF + (sp + 1) * KV_V_SZ]
                        .rearrange("(p x) -> p x", p=P))

            # attention: for each head PAIR, stream m tiles; the two 64-row
            # score matmuls go to different PE row groups (concurrent), one
            # Exp covers both heads, and attn@V uses fp8 DoubleRow over
            # m-pair et tiles.  et tile layout: [m0:h0|h1, m1:h0|h1].
            pend = []          # queued (et tile, pair index mp, head h, hh)
            norm_pend = []     # queued (h, rden_b) normalize chains
            pats = {}

            def flush_pat2():
                for et_t, mp, h, hh in pend:
                    va = v_aug[mp][:].rearrange("p (s x) -> p s x", s=2)
                    e4 = et_t[:].rearrange("p (mm j x) -> p mm j x",
                                           j=2, x=NPC)
                    nc.tensor.matmul(
                        pats[h][:, :],
                        lhsT=va[:, :, h * VBLK:h * VBLK + DH + 1],
                        rhs=e4[:, :, hh, :],
                        start=(mp == 0), stop=(mp == NGRP - 1),
                        perf_mode=DR)
                del pend[:]

            def flush_norm():
                for h, rden_b in norm_pend:
                    rbp = ps_misc([DH, NPC], f32)
                    nc.tensor.matmul(rbp[0:DH, :], lhsT=ones64_b[0:1, :],
                                     rhs=rden_b[:], start=True, stop=True)
                    rdb = work.tile([DH, NPC], f32, name="rdb")
                    nc.vector.tensor_copy(rdb[:], rbp[0:DH, :])
                    nc.vector.scalar_tensor_tensor(
                        attn_h[h][:], pats[h][0:DH, :], 1.0, rdb[:],
                        op0=ALU.mult, op1=ALU.mult)
                del norm_pend[:]

            escale = float(1.0 / (np.sqrt(DH) * GQKV * GQKV))
            for g in range(2):
                pats[2 * g] = ps_pat()
                pats[2 * g + 1] = ps_pat()
                et_t = None
                for m in range(NST):
                    if m % 2 == 0:
                        et_t = work.tile([P, 4 * NPC], fp8, name="et",
                                         bufs=3)
                    sps = ps3()
                    for hh in range(2):
                        r = hh * DH
                        nc.tensor.matmul(
                            sps[:, hh * NPC:(hh + 1) * NPC],
                            lhsT=kT_full[g][r:r + DH, m * P:(m + 1) * P],
                            rhs=qT[g][r:r + DH, :], start=True, stop=True)
                    if m == 2:
                        flush_norm()
                    nc.scalar.activation(
                        et_t[:, (m % 2) * 2 * NPC:((m % 2) + 1) * 2 * NPC],
                        sps[:], AF.Exp, scale=escale)
                    flush_pat2()
                    if m % 2 == 1:
                        mp = m // 2
                        pend.append((et_t, mp, 2 * g, 0))
                        pend.append((et_t, mp, 2 * g + 1, 1))
                flush_pat2()
                # normalize part A (DVE): denominator reciprocals
                for hh in range(2):
                    h = 2 * g + hh
                    dens = work.tile([1, NPC], f32, name="dens")
                    rden = work.tile([1, NPC], f32, name="rden")
                    rden_b = work.tile([1, NPC], bf16, name="rden_b")
                    nc.vector.tensor_copy(dens[:], pats[h][DH:DH + 1, :])
                    nc.vector.reciprocal_approx_fast(rden[:], dens[:])
                    nc.vector.tensor_copy(rden_b[:], rden[:])
                    norm_pend.append((h, rden_b))
            flush_norm()

            # out-proj (+bias*64) with relu, scaled by dinv/64*64, fp8 out
            for s in range(NSTRIP):
                ops = psum.tile([P, HID], f32, name="ps3", bufs=2)
                for h in range(NH):
                    nc.tensor.matmul(ops[:],
                                     lhsT=attn_h[h][:, s * P:(s + 1) * P],
                                     rhs=opwH[h][:], start=(h == 0),
                                     stop=False)
                nc.tensor.matmul(ops[:], lhsT=ones_row_b[0:1, :],
                                 rhs=opb[:], start=False, stop=True)
                nc.vector.tensor_scalar(x_n[s][:], ops[:], 0.0,
                                        dinv_nm[:, s:s + 1],
                                        op0=ALU.max, op1=ALU.mult)

        def ag_x(idx):
            xloc, xglob = x_bufs[idx]
            for s in range(NSTRIP):
                nc.sync.dma_start(xloc[s * P:(s + 1) * P, :], x_n[s][:])
            nc.gpsimd.collective_compute(
                "AllGather", ALU.bypass, replica_groups=RG,
                ins=[xloc[:, :]], outs=[xglob[:, :, :]])
            for c in range(NCORES):
                for s in range(NSTRIP):
                    t = c * NSTRIP + s
                    nc.sync.dma_start(
                        xN_all[:, t * HID:(t + 1) * HID],
                        xglob[c, s * P:(s + 1) * P, :])

        # ---------------- network ----------------
        def evict_h(fo, ps, biasC):
            nc.vector.tensor_scalar(xT_own[fo][:], ps,
                                    biasC[:, fo:fo + 1], None, op0=ALU.add)

        gcn_layer(x0s_all, DIN, W1t, dinv_bc8, b1C, 2, evict_h)
        mha(0)
        ag_x(0)

        gcn_layer(xN_all, HID, W2t, dinv_bc64, b2C, 2, evict_h)
        mha(1)
        ag_x(1)

        def evict_out(fo, ps, biasC):
            nc.scalar.activation(out_f[:], ps, AF.Sigmoid,
                                 bias=biasC[:, 0:1])
            nc.sync.dma_start(d_out[:, :], out_f[:])

        gcn_layer(xN_all, HID, W3t, dinv_bc64, b3C, 1, evict_out)


# ----------------------------------------------------------------------------
# Host-side prep: pure index manipulation / layout / dtype casts.
# ----------------------------------------------------------------------------

def _prep_inputs(node_features, edge_index, edge_weight, W1, b1, W2, b2,
                 W3, b3, in_proj_w, in_proj_b, out_proj_w, out_proj_b):
    bfl = mybir.dt.np(bf16)
    f8 = mybir.dt.np(fp8)
    rows = np.concatenate([np.asarray(edge_index[0], np.int64),
                           np.arange(N, dtype=np.int64)])
    cols = np.concatenate([np.asarray(edge_index[1], np.int64),
                           np.arange(N, dtype=np.int64)])
    w = np.concatenate([np.asarray(edge_weight, np.float32),
                        np.ones(N, np.float32)])
    A = np.zeros((N, N), np.float32)
    np.add.at(A, (rows, cols), w)
    A8 = A.astype(f8)

    x0 = np.asarray(node_features, np.float32).astype(bfl)
    x0p = np.ascontiguousarray(
        x0.reshape(NST, P, DIN).transpose(1, 0, 2).reshape(P, NST * DIN))

    asf = lambda a: np.ascontiguousarray(np.asarray(a, np.float32))
    asb = lambda a: np.ascontiguousarray(np.asarray(a, np.float32)
                                         .astype(bfl))
    common = {
        "x0p": x0p,
        "W1": asb(W1),
        "W2": asb(W2),
        "W3": asb(W3),
        "b1C": asf(np.asarray(b1, np.float32).reshape(2, P).T),
        "b2C": asf(np.asarray(b2, np.float32).reshape(2, P).T),
        "b3C": asf(np.asarray(b3, np.float32).reshape(1, P).T),
        "ipw": asb(np.asarray(in_proj_w, np.float32).T),
        "ipbC64": asf(np.asarray(in_proj_b, np.float32).reshape(6, P).T
                      * GQKV),
        "opwH": asb(np.asarray(out_proj_w, np.float32).T),
        "opb64": asb(np.asarray(out_proj_b, np.float32).reshape(1, HID)
                     * GQKV),
    }
    in_maps = []
    for c in range(NCORES):
        m = dict(common)
        Ac = A8[:, c * NPC:(c + 1) * NPC]
        m["ATp"] = np.ascontiguousarray(
            Ac.reshape(NST, P, NPC).transpose(1, 0, 2)
            .reshape(P, NST * NPC))
        in_maps.append(m)
    return in_maps


_CACHE = {}
TRACE = False
LAST_RESULTS = None


def _get_program():
    if "prog" not in _CACHE:
        _CACHE["prog"] = _build_program()
    return _CACHE["prog"]


def kernel(**inputs):
    global LAST_RESULTS
    inputs = {k: np.asarray(v) for k, v in inputs.items()}
    in_maps = _prep_inputs(**inputs)
    nc = _get_program()
    res = bass_utils.run_bass_kernel_spmd(nc, in_maps,
                                          core_ids=list(range(NCORES)),
                                          trace=TRACE)
    LAST_RESULTS = res
    out = np.concatenate(
        [np.asarray(res.results[c]["out"]).T for c in range(NCORES)], axis=0)
    return out.astype(np.float32)


# revision 22
# speedup vs baseline: 1.1221x; 1.1127x over previous
"""Trainium2 Bass kernel for ComplexGCN (3x GCNConv + 2x MHA), 8-core SPMD.

Strategy (v3): shard destination nodes across 8 cores (512 nodes/core).
The unnormalized dense adjacency shard A^T [4096 src, 512 dst] is assembled
on the host (pure index-driven scatter of the edge list, duplicates
coalesced, self loops added), packed partition-major for wide DMA lines,
and shipped as fp8; all model math runs on device:

  deg = column sums of A^T (ones-stationary matmuls) -> AllGather ->
  dinv = 1/sqrt(deg).  GCN aggregation runs feature-major with fp8
  DoubleRow matmuls (psum[feat,dst] += x_pair^T @ A^T_pair), the symmetric
  normalization folded into input/eviction scales, and the dense W
  transform applied AFTER aggregation ((A^T X) W == A^T (X W)).  MHA
  computes q/k/v for own nodes only (fp8, gain 64), AllGathers k
  (feature-major) and v (node-major pairs with a ones column per head for
  the softmax denominator), then per head streams score matmuls ->
  grouped Exp on the ACT engine (2 PSUM banks per activation, fp8 out) ->
  fp8 DoubleRow attn@V accumulation, software-pipelined so the in-order
  tensor queue never stalls on ACT.  Per-head softmax denominators use
  reciprocal_approx_fast; out-proj contracts per head (64 rows).  All
  psum evictions run on DVE so ACT does (almost) nothing but Exp.

Matmuls are emitted back-to-back so the PE array's HAM clock gate stays
at 2.4 GHz.
"""

import numpy as np

import concourse.bass as bass
import concourse.bacc as bacc
import concourse.mybir as mybir
import concourse.tile as tile
from concourse import bass_utils
from concourse.masks import make_identity

P = 128
N = 4096
NCORES = 8
NPC = N // NCORES          # 512 dst nodes per core
NST = N // P               # 32 src tiles
NSTRIP = NPC // P          # 4 own strips
DIN = 256
HID = 256
DOUT = 128
NH = 4
DH = 64

GX0 = 8.0                  # fp8 gain on dinv-scaled x0
GQKV = 64.0                # fp8 gain on q/k/v

f32 = mybir.dt.float32
bf16 = mybir.dt.bfloat16
fp8 = mybir.dt.float8e4
AF = mybir.ActivationFunctionType
ALU = mybir.AluOpType
DR = mybir.MatmulPerfMode.DoubleRow
RG = [list(range(NCORES))]

VBLK = 68                  # per-head block in v tiles: 64 v + 1 one + 3 pad
VW = NH * VBLK             # 272 cols per m-tile
KV_V_SZ = P * 2 * VW       # one v pair-tile [128, 544]
GRP = 2                    # score tiles per Exp activation (= one DR pair)
NGRP = NST // GRP


def _build_program():
    nc = bacc.Bacc("TRN2", target_bir_lowering=False, debug=False,
                   num_devices=NCORES)

    # ---- external I/O ----
    d_AT = nc.dram_tensor("ATp", [P, NST * NPC], fp8, kind="ExternalInput")
    d_x0 = nc.dram_tensor("x0p", [P, NST * DIN], bf16, kind="ExternalInput")
    d_W1 = nc.dram_tensor("W1", [DIN, HID], bf16, kind="ExternalInput")
    d_W2 = nc.dram_tensor("W2", [HID, HID], bf16, kind="ExternalInput")
    d_W3 = nc.dram_tensor("W3", [HID, DOUT], bf16, kind="ExternalInput")
    d_b1 = nc.dram_tensor("b1C", [P, 2], f32, kind="ExternalInput")
    d_b2 = nc.dram_tensor("b2C", [P, 2], f32, kind="ExternalInput")
    d_b3 = nc.dram_tensor("b3C", [P, 1], f32, kind="ExternalInput")
    d_ipw = nc.dram_tensor("ipw", [HID, 3 * HID], bf16, kind="ExternalInput")
    d_ipb = nc.dram_tensor("ipbC64", [P, 6], f32, kind="ExternalInput")
    d_opwH = nc.dram_tensor("opwH", [NH * DH, HID], bf16,
                            kind="ExternalInput")
    d_opb = nc.dram_tensor("opb64", [1, HID], bf16, kind="ExternalInput")
    d_out = nc.dram_tensor("out", [DOUT, NPC], f32, kind="ExternalOutput")

    # ---- internal DRAM for collectives ----
    d_dmy = nc.dram_tensor("dmy_loc", [NCORES], f32)
    d_dmyg = nc.dram_tensor("dmy_glob", [NCORES * NCORES], f32,
                            addr_space="Shared")
    d_degl = nc.dram_tensor("deg_loc", [NPC], f32)
    d_degg = nc.dram_tensor("deg_glob", [N], f32, addr_space="Shared")
    KV_TOT = 2 * P * NPC + 2 * KV_V_SZ
    kv_bufs = []
    for i in range(2):
        kl = nc.dram_tensor(f"kv{i}_loc", [KV_TOT], fp8)
        kg = nc.dram_tensor(f"kv{i}_glob", [NCORES, KV_TOT], fp8,
                            addr_space="Shared")
        kv_bufs.append((kl, kg))
    x_bufs = []
    for i in range(2):
        loc = nc.dram_tensor(f"x{i}_loc", [NPC, HID], fp8)
        glob = nc.dram_tensor(f"x{i}_glob", [NCORES, NPC, HID], fp8,
                              addr_space="Shared")
        x_bufs.append((loc, glob))

    with tile.TileContext(nc) as tc:
        _emit(nc, tc, d_AT, d_x0, d_W1, d_W2, d_W3, d_b1, d_b2, d_b3,
              d_ipw, d_ipb, d_opwH, d_opb, d_out,
              d_degl, d_degg, kv_bufs, x_bufs, d_dmy, d_dmyg)
    nc.compile()
    return nc


def _emit(nc, tc, d_AT, d_x0, d_W1, d_W2, d_W3, d_b1, d_b2, d_b3,
          d_ipw, d_ipb, d_opwH, d_opb, d_out,
          d_degl, d_degg, kv_bufs, x_bufs, d_dmy, d_dmyg):
    from contextlib import ExitStack
    ctx = ExitStack()
    with ctx:
        const = ctx.enter_context(tc.tile_pool(name="const", bufs=1))
        big = ctx.enter_context(tc.tile_pool(name="big", bufs=1))
        work = ctx.enter_context(tc.tile_pool(name="work", bufs=2))
        psum = ctx.enter_context(tc.tile_pool(name="psum", bufs=1,
                                              space="PSUM"))

        # One psum tag ("ps3", 2 banks x 3 bufs) for everything transient plus
        # a 2-buf accumulator tag ("pat") = exactly 8 banks.  bufs=3 gives the
        # co-critical tensor/ACT score->exp->pat stream enough slack that
        # cross-engine semaphore latency stays off the critical path.
        def ps3():
            return psum.tile([P, GRP * NPC], f32, name="ps3", bufs=3)

        def ps_pat():
            return psum.tile([DH + 1, NPC], f32, name="pat", bufs=2)

        def ps_misc(shape, dt):
            return psum.tile(shape, dt, name="ps3", bufs=3)

        # ---------------- constants ----------------
        ident_f = const.tile([P, P], f32, name="ident_f")
        make_identity(nc, ident_f[:])
        ident_b = const.tile([P, P], bf16, name="ident_b")
        make_identity(nc, ident_b[:])
        ones_col8 = const.tile([P, 1], fp8, name="ones_col8")
        nc.vector.memset(ones_col8[:], 1.0)
        ones_row_b = const.tile([1, P], bf16, name="ones_row_b")
        nc.vector.memset(ones_row_b[:], 1.0)
        ones_row_f = const.tile([1, P], f32, name="ones_row_f")
        nc.vector.memset(ones_row_f[:], 1.0)
        ones64_b = const.tile([1, DH], bf16, name="ones64_b")
        nc.vector.memset(ones64_b[:], 1.0)

        # ---------------- weight loads (pre-cast on host) ----------------
        def load(dram, shape, dt, tag):
            t = const.tile(shape, dt, name=tag)
            nc.sync.dma_start(t[:], dram)
            return t

        W1t = [load(d_W1[k * P:(k + 1) * P, :], [P, HID], bf16, f"W1t{k}")
               for k in range(2)]
        W2t = [load(d_W2[k * P:(k + 1) * P, :], [P, HID], bf16, f"W2t{k}")
               for k in range(2)]
        W3t = [load(d_W3[k * P:(k + 1) * P, :], [P, DOUT], bf16, f"W3t{k}")
               for k in range(2)]
        ipw = [load(d_ipw[k * P:(k + 1) * P, :], [P, 3 * HID], bf16,
                    f"ipw{k}") for k in range(2)]
        opwH = [load(d_opwH[h * DH:(h + 1) * DH, :], [DH, HID], bf16,
                     f"opwH{h}") for h in range(NH)]
        opb = load(d_opb[:, :], [1, HID], bf16, "opb")
        b1C = load(d_b1[:, :], [P, 2], f32, "b1C")
        b2C = load(d_b2[:, :], [P, 2], f32, "b2C")
        b3C = load(d_b3[:, :], [P, 1], f32, "b3C")
        ipbC = load(d_ipb[:, :], [P, 6], f32, "ipbC")

        # ---------------- big persistent tiles ----------------
        AT_all = big.tile([P, NST * NPC], fp8, name="AT_all")
        x0_all = big.tile([P, NST * DIN], bf16, name="x0_all")
        x0s_all = big.tile([P, NST * DIN], fp8, name="x0s_all")
        xN_all = big.tile([P, NST * HID], fp8, name="xN_all")
        kT_full = [big.tile([P, N], fp8, name=f"kTf{g}") for g in range(2)]
        v_aug = [big.tile([P, 2 * VW], fp8, name=f"vaug{mp}")
                 for mp in range(NST // 2)]
        qT = [big.tile([P, NPC], fp8, name=f"qT{g}") for g in range(2)]
        ktmp = [big.tile([P, NPC], fp8, name=f"ktmp{g}") for g in range(2)]
        vT_own = [big.tile([P, NPC], bf16, name=f"vTo{g}") for g in range(2)]
        vaug_own = [big.tile([P, 2 * VW], fp8, name=f"vaugo{sp}")
                    for sp in range(2)]
        xTagg = [big.tile([P, NPC], bf16, name=f"xTagg{k}") for k in range(2)]
        xT_own = [big.tile([P, NPC], bf16, name=f"xTo{k}") for k in range(2)]
        attn_h = [big.tile([DH, NPC], bf16, name=f"attn{h}")
                  for h in range(NH)]
        x_n = [big.tile([P, HID], fp8, name=f"x_n{s}") for s in range(NSTRIP)]
        dinv_bc8 = big.tile([P, NPC], f32, name="dinv_bc8")
        dinv_bc64 = big.tile([P, NPC], f32, name="dinv_bc64")
        out_f = big.tile([DOUT, NPC], f32, name="out_f")

        deg_row = const.tile([1, NPC], f32, name="deg_row")
        sq_row = const.tile([1, NPC], f32, name="sq_row")
        dinv_row = const.tile([1, NPC], f32, name="dinv_row")
        deg_all = const.tile([P, NST], f32, name="deg_all")
        sq_all = const.tile([P, NST], f32, name="sq_all")
        dinv_all8 = const.tile([P, NST], f32, name="dinv_all8")
        dinv_nm = const.tile([P, NSTRIP], f32, name="dinv_nm")

        # ---------------- input DMAs (wide, packed on host) --------------
        # A first: the degree reduction (and its AllGather) gates GCN1.
        NCHUNK = 8
        for ch in range(NCHUNK):
            w = NST * NPC // NCHUNK
            nc.sync.dma_start(AT_all[:, ch * w:(ch + 1) * w],
                              d_AT[:, ch * w:(ch + 1) * w])
        for ch in range(4):
            w = NST * DIN // 4
            nc.sync.dma_start(x0_all[:, ch * w:(ch + 1) * w],
                              d_x0[:, ch * w:(ch + 1) * w])

        def at_t(t):
            return AT_all[:, t * NPC:(t + 1) * NPC]

        # warm up the CC mesh path while inputs stream in
        nc.gpsimd.collective_compute(
            "AllGather", ALU.bypass, replica_groups=RG,
            ins=[d_dmy[:]], outs=[d_dmyg[:]])

        # ---------------- degree + dinv ----------------
        dps = ps_misc([1, NPC], f32)
        for t in range(NST):
            nc.tensor.matmul(dps[0:1, :], lhsT=ones_col8[:, 0:1],
                             rhs=at_t(t), start=(t == 0),
                             stop=(t == NST - 1))
        nc.vector.tensor_copy(deg_row[:], dps[0:1, :])
        # scalar-engine DMA queue: dodges head-of-line blocking behind the
        # big input chunks on the sync queues
        nc.scalar.dma_start(
            d_degl.ap().rearrange("(a b) -> a b", a=1), deg_row[:])
        nc.gpsimd.collective_compute(
            "AllGather", ALU.bypass, replica_groups=RG,
            ins=[d_degl[:]], outs=[d_degg[:]])
        nc.scalar.dma_start(deg_all[:],
                            d_degg.ap().rearrange("(t p) -> p t", p=P))
        nc.scalar.sqrt(sq_all[:], deg_all[:])
        nc.vector.reciprocal(dinv_all8[:], sq_all[:])
        nc.vector.tensor_scalar(dinv_all8[:], dinv_all8[:], GX0, None,
                                op0=ALU.mult)
        # own-node dinv row (local, no collective needed)
        nc.scalar.sqrt(sq_row[:], deg_row[:])
        nc.vector.reciprocal_approx_fast(dinv_row[:], sq_row[:])
        # own-node dinv p-major (for out-proj folds)
        tp = ps_misc([P, NSTRIP], f32)
        for s in range(NSTRIP):
            nc.tensor.transpose(tp[:, s:s + 1],
                                dinv_row[0:1, s * P:(s + 1) * P],
                                ident_f[0:1, 0:1])
        nc.vector.tensor_copy(dinv_nm[:], tp[:, 0:NSTRIP])
        # broadcast own dinv over partitions (f-major column scale)
        bcp = ps_misc([P, NPC], f32)
        for s in range(NSTRIP):
            nc.tensor.matmul(bcp[:, s * P:(s + 1) * P],
                             lhsT=ones_row_f[0:1, :],
                             rhs=dinv_row[0:1, s * P:(s + 1) * P],
                             start=True, stop=True)
        nc.vector.tensor_scalar(dinv_bc8[:], bcp[:], 1.0 / GX0, None,
                                op0=ALU.mult)
        nc.vector.tensor_scalar(dinv_bc64[:], bcp[:], 1.0 / GQKV, None,
                                op0=ALU.mult)

        # x0 -> fp8, scaled by GX0 * dinv[src]
        for t in range(NST):
            nc.vector.tensor_scalar(
                x0s_all[:, t * DIN:(t + 1) * DIN],
                x0_all[:, t * DIN:(t + 1) * DIN],
                dinv_all8[:, t:t + 1], None, op0=ALU.mult)

        # ---------------- helpers ----------------
        def gcn_layer(x_all, xw, Wt, bc, biasC, n_fo, evict):
            """xTagg = bc * (x^T A^T) via fp8 DoubleRow; W^T @ xTagg -> evict."""
            aps = ps3()
            for tp2 in range(NST // 2):
                xpair = x_all[:, tp2 * 2 * xw:(tp2 + 1) * 2 * xw] \
                    .rearrange("p (s x) -> p s x", s=2)
                apair = AT_all[:, tp2 * 2 * NPC:(tp2 + 1) * 2 * NPC] \
                    .rearrange("p (s x) -> p s x", s=2)
                for k in range(2):
                    nc.tensor.matmul(aps[:, k * NPC:(k + 1) * NPC],
                                     lhsT=xpair[:, :, k * P:(k + 1) * P],
                                     rhs=apair[:, :, :],
                                     start=(tp2 == 0),
                                     stop=(tp2 == NST // 2 - 1),
                                     perf_mode=DR)
            for k in range(2):
                nc.vector.scalar_tensor_tensor(
                    xTagg[k][:], aps[:, k * NPC:(k + 1) * NPC], 1.0, bc[:],
                    op0=ALU.mult, op1=ALU.mult)
            wps = ps3()
            for fo in range(n_fo):
                for fi in range(2):
                    nc.tensor.matmul(wps[:, fo * NPC:(fo + 1) * NPC],
                                     lhsT=Wt[fi][:, fo * P:(fo + 1) * P],
                                     rhs=xTagg[fi][:], start=(fi == 0),
                                     stop=(fi == 1))
            for fo in range(n_fo):
                evict(fo, wps[:, fo * NPC:(fo + 1) * NPC], biasC)

        def mha(idx):
            """xT_own (f-major) -> x_n (n-major fp8 = relu(out) * dinv)."""
            kvloc, kvglob = kv_bufs[idx]
            V_OFF = 2 * P * NPC
            # qkv for own nodes (gain 64): k, v first (feed the AG), then q
            jorder = [2, 3, 4, 5, 0, 1]
            dests = {0: qT[0], 1: qT[1], 2: ktmp[0], 3: ktmp[1],
                     4: vT_own[0], 5: vT_own[1]}
            qps = None
            for jj, j in enumerate(jorder):
                if jj % GRP == 0:
                    qps = ps3()
                sl = qps[:, (jj % GRP) * NPC:((jj % GRP) + 1) * NPC]
                for fi in range(2):
                    nc.tensor.matmul(sl, lhsT=ipw[fi][:, j * P:(j + 1) * P],
                                     rhs=xT_own[fi][:], start=(fi == 0),
                                     stop=(fi == 1))
                nc.vector.tensor_scalar(dests[j][:], sl, GQKV,
                                        ipbC[:, j:j + 1],
                                        op0=ALU.mult, op1=ALU.add)
                if j == 2 or j == 3:
                    g = j - 2
                    nc.sync.dma_start(
                        kvloc[g * P * NPC:(g + 1) * P * NPC]
                        .rearrange("(p x) -> p x", p=P), ktmp[g][:])
            # v: transpose to node-major augmented fp8 pair layout
            for k2 in range(2):
                for s in range(NSTRIP):
                    vtp = ps_misc([P, P], bf16)
                    nc.tensor.transpose(vtp[:],
                                        vT_own[k2][:, s * P:(s + 1) * P],
                                        ident_b[:])
                    sp, half = s // 2, (s % 2) * VW
                    for hh in range(2):
                        h = 2 * k2 + hh
                        o = half + h * VBLK
                        nc.vector.tensor_copy(
                            vaug_own[sp][:, o:o + DH],
                            vtp[:, hh * DH:(hh + 1) * DH])
            for sp in range(2):
                va = vaug_own[sp][:].rearrange("p (b x) -> p b x", x=VBLK)
                nc.vector.memset(va[:, :, DH:DH + 1], 1.0)
                nc.sync.dma_start(
                    kvloc[V_OFF + sp * KV_V_SZ:V_OFF + (sp + 1) * KV_V_SZ]
                    .rearrange("(p x) -> p x", p=P), vaug_own[sp][:])
            nc.gpsimd.collective_compute(
                "AllGather", ALU.bypass, replica_groups=RG,
                ins=[kvloc[:]], outs=[kvglob[:, :]])
            # unpack
            for c in range(NCORES):
                for g in range(2):
                    nc.sync.dma_start(
                        kT_full[g][:, c * NPC:(c + 1) * NPC],
                        kvglob[c, g * P * NPC:(g + 1) * P * NPC]
                        .rearrange("(p x) -> p x", p=P))
                for sp in range(2):
                    nc.sync.dma_start(
                        v_aug[c * 2 + sp][:],
                        kvglob[c, V_OFF + sp * KV_V_SZ:
                               V_OFF + (sp + 1) * KV_V_SZ]
                        .rearrange("(p x) -> p x", p=P))

            # attention: for each head PAIR, stream m tiles; the two 64-row
            # score matmuls go to different PE row groups (concurrent), one
            # Exp covers both heads, and attn@V uses fp8 DoubleRow over
            # m-pair et tiles.  et tile layout: [m0:h0|h1, m1:h0|h1].
            pend = []          # queued (et tile, pair index mp, head h, hh)
            norm_pend = []     # queued (h, rden_b) normalize chains
            pats = {}

            def flush_pat2():
                for et_t, mp, h, hh in pend:
                    va = v_aug[mp][:].rearrange("p (s x) -> p s x", s=2)
                    e4 = et_t[:].rearrange("p (mm j x) -> p mm j x",
                                           j=2, x=NPC)
                    nc.tensor.matmul(
                        pats[h][:, :],
                        lhsT=va[:, :, h * VBLK:h * VBLK + DH + 1],
                        rhs=e4[:, :, hh, :],
                        start=(mp == 0), stop=(mp == NGRP - 1),
                        perf_mode=DR)
                del pend[:]

            def flush_norm():
                for h, rden_b in norm_pend:
                    rbp = ps_misc([DH, NPC], f32)
                    nc.tensor.matmul(rbp[0:DH, :], lhsT=ones64_b[0:1, :],
                                     rhs=rden_b[:], start=True, stop=True)
                    rdb = work.tile([DH, NPC], f32, name="rdb")
                    nc.vector.tensor_copy(rdb[:], rbp[0:DH, :])
                    nc.vector.scalar_tensor_tensor(
                        attn_h[h][:], pats[h][0:DH, :], 1.0, rdb[:],
                        op0=ALU.mult, op1=ALU.mult)
                del norm_pend[:]

            escale = float(1.0 / (np.sqrt(DH) * GQKV * GQKV))
            for g in range(2):
                pats[2 * g] = ps_pat()
                pats[2 * g + 1] = ps_pat()
                et_t = None
                for m in range(NST):
                    if m % 2 == 0:
                        et_t = work.tile([P, 4 * NPC], fp8, name="et",
                                         bufs=3)
                    sps = ps3()
                    for hh in range(2):
                        r = hh * DH
                        nc.tensor.matmul(
                            sps[:, hh * NPC:(hh + 1) * NPC],
                            lhsT=kT_full[g][r:r + DH, m * P:(m + 1) * P],
                            rhs=qT[g][r:r + DH, :], start=True, stop=True)
                    if m == 2:
                        flush_norm()
                    nc.scalar.activation(
                        et_t[:, (m % 2) * 2 * NPC:((m % 2) + 1) * 2 * NPC],
                        sps[:], AF.Exp, scale=escale)
                    flush_pat2()
                    if m % 2 == 1:
                        mp = m // 2
                        pend.append((et_t, mp, 2 * g, 0))
                        pend.append((et_t, mp, 2 * g + 1, 1))
                flush_pat2()
                # normalize part A (DVE): denominator reciprocals
                for hh in range(2):
                    h = 2 * g + hh
                    dens = work.tile([1, NPC], f32, name="dens")
                    rden = work.tile([1, NPC], f32, name="rden")
                    rden_b = work.tile([1, NPC], bf16, name="rden_b")
                    nc.vector.tensor_copy(dens[:], pats[h][DH:DH + 1, :])
                    nc.vector.reciprocal_approx_fast(rden[:], dens[:])
                    nc.vector.tensor_copy(rden_b[:], rden[:])
                    norm_pend.append((h, rden_b))
            flush_norm()

            # out-proj (+bias*64) with relu, scaled by dinv/64*64, fp8 out
            for s in range(NSTRIP):
                ops = psum.tile([P, HID], f32, name="ps3", bufs=3)
                for h in range(NH):
                    nc.tensor.matmul(ops[:],
                                     lhsT=attn_h[h][:, s * P:(s + 1) * P],
                                     rhs=opwH[h][:], start=(h == 0),
                                     stop=False)
                nc.tensor.matmul(ops[:], lhsT=ones_row_b[0:1, :],
                                 rhs=opb[:], start=False, stop=True)
                nc.vector.tensor_scalar(x_n[s][:], ops[:], 0.0,
                                        dinv_nm[:, s:s + 1],
                                        op0=ALU.max, op1=ALU.mult)

        def ag_x(idx):
            xloc, xglob = x_bufs[idx]
            for s in range(NSTRIP):
                nc.sync.dma_start(xloc[s * P:(s + 1) * P, :], x_n[s][:])
            nc.gpsimd.collective_compute(
                "AllGather", ALU.bypass, replica_groups=RG,
                ins=[xloc[:, :]], outs=[xglob[:, :, :]])
            for c in range(NCORES):
                for s in range(NSTRIP):
                    t = c * NSTRIP + s
                    nc.sync.dma_start(
                        xN_all[:, t * HID:(t + 1) * HID],
                        xglob[c, s * P:(s + 1) * P, :])

        # ---------------- network ----------------
        def evict_h(fo, ps, biasC):
            nc.vector.tensor_scalar(xT_own[fo][:], ps,
                                    biasC[:, fo:fo + 1], None, op0=ALU.add)

        gcn_layer(x0s_all, DIN, W1t, dinv_bc8, b1C, 2, evict_h)
        mha(0)
        ag_x(0)

        gcn_layer(xN_all, HID, W2t, dinv_bc64, b2C, 2, evict_h)
        mha(1)
        ag_x(1)

        def evict_out(fo, ps, biasC):
            nc.scalar.activation(out_f[:], ps, AF.Sigmoid,
                                 bias=biasC[:, 0:1])
            nc.sync.dma_start(d_out[:, :], out_f[:])

        gcn_layer(xN_all, HID, W3t, dinv_bc64, b3C, 1, evict_out)


# ----------------------------------------------------------------------------
# Host-side prep: pure index manipulation / layout / dtype casts.
# ----------------------------------------------------------------------------

def _prep_inputs(node_features, edge_index, edge_weight, W1, b1, W2, b2,
                 W3, b3, in_proj_w, in_proj_b, out_proj_w, out_proj_b):
    bfl = mybir.dt.np(bf16)
    f8 = mybir.dt.np(fp8)
    rows = np.concatenate([np.asarray(edge_index[0], np.int64),
                           np.arange(N, dtype=np.int64)])
    cols = np.concatenate([np.asarray(edge_index[1], np.int64),
                           np.arange(N, dtype=np.int64)])
    w = np.concatenate([np.asarray(edge_weight, np.float32),
                        np.ones(N, np.float32)])
    A = np.zeros((N, N), np.float32)
    np.add.at(A, (rows, cols), w)
    A8 = A.astype(f8)

    x0 = np.asarray(node_features, np.float32).astype(bfl)
    x0p = np.ascontiguousarray(
        x0.reshape(NST, P, DIN).transpose(1, 0, 2).reshape(P, NST * DIN))

    asf = lambda a: np.ascontiguousarray(np.asarray(a, np.float32))
    asb = lambda a: np.ascontiguousarray(np.asarray(a, np.float32)
                                         .astype(bfl))
    common = {
        "x0p": x0p,
        "W1": asb(W1),
        "W2": asb(W2),
        "W3": asb(W3),
        "b1C": asf(np.asarray(b1, np.float32).reshape(2, P).T),
        "b2C": asf(np.asarray(b2, np.float32).reshape(2, P).T),
        "b3C": asf(np.asarray(b3, np.float32).reshape(1, P).T),
        "ipw": asb(np.asarray(in_proj_w, np.float32).T),
        "ipbC64": asf(np.asarray(in_proj_b, np.float32).reshape(6, P).T
                      * GQKV),
        "opwH": asb(np.asarray(out_proj_w, np.float32).T),
        "opb64": asb(np.asarray(out_proj_b, np.float32).reshape(1, HID)
                     * GQKV),
    }
    in_maps = []
    for c in range(NCORES):
        m = dict(common)
        Ac = A8[:, c * NPC:(c + 1) * NPC]
        m["ATp"] = np.ascontiguousarray(
            Ac.reshape(NST, P, NPC).transpose(1, 0, 2)
            .reshape(P, NST * NPC))
        in_maps.append(m)
    return in_maps


_CACHE = {}
TRACE = False
LAST_RESULTS = None


def _get_program():
    if "prog" not in _CACHE:
        _CACHE["prog"] = _build_program()
    return _CACHE["prog"]


def kernel(**inputs):
    global LAST_RESULTS
    inputs = {k: np.asarray(v) for k, v in inputs.items()}
    in_maps = _prep_inputs(**inputs)
    nc = _get_program()
    res = bass_utils.run_bass_kernel_spmd(nc, in_maps,
                                          core_ids=list(range(NCORES)),
                                          trace=TRACE)
    LAST_RESULTS = res
    out = np.concatenate(
        [np.asarray(res.results[c]["out"]).T for c in range(NCORES)], axis=0)
    return out.astype(np.float32)
